# revision 12
# baseline (speedup 1.0000x reference)
"""Additive attention (B=8, Lq=Lk=H=D=256) on 8 trn2 NeuronCores.

Data-parallel over batch: core b computes batch b.
Per core:
  qprojT[h,q] = sum_d W_q[h,d] q[q,d]   (PE)
  kprojT[h,k] = sum_d W_k[h,d] k[k,d]   (PE)
  for each k: scores[q,k] = sum_h W_v[h] * tanh(qprojT[h,q] + kprojT[h,k])
    - broadcast-add on DVE (tensor_scalar, per-partition scalar = kprojT[:,k])
    - tanh on ACT, batched 16 k-values per instruction ([128, 4096]), bf16 out
    - H-reduction on PE: F-chunk [h,q] stationary, W_v chunk [h,1] moving,
      N=1 column accumulated into a scores[q,k] PSUM tile
  PE-transpose scores -> scoresT[k,q]
  mask: rows k >= valid_len multiplied by 0 (constant row -> uniform weights
    after the softmax over q, exactly the reference's masked softmax)
  softmax over q (free axis) per k row; out = attnT.T @ values (PE)
"""

import sys

sys.path.insert(0, "/opt/trn_rl_repo")

import numpy as np

import concourse.bass as bass
import concourse.mybir as mybir
from concourse.tile import TileContext
from concourse.bass_utils import run_bass_kernel_spmd

F32 = mybir.dt.float32
BF16 = mybir.dt.bfloat16
FP8 = mybir.dt.float8e4
AF = mybir.ActivationFunctionType
AX = mybir.AxisListType
OP = mybir.AluOpType

B, LQ, LK, D, H = 8, 256, 256, 256, 256
KBLK = 16  # k-values batched per ACT tanh instruction


def _split_multiwait(nc):
    """The installed walrus accepts only one sync-wait per CTRL instruction,
    but TileContext's tail drain is emitted after tile_legalize and can carry
    several. Split extras into single-wait drains placed just before it."""
    for f in nc.m.functions:
        for bb in f.blocks:
            newlist = []
            changed = False
            for ins in bb.instructions:
                si = ins.sync_info
                if si is not None and si.on_wait and len(si.on_wait) > 1:
                    waits = list(si.on_wait)
                    for i, w in enumerate(waits[:-1]):
                        d = mybir.InstDrain(
                            name=f"{ins.name}_w{i}",
                            ins=[],
                            outs=[],
                            sync_info=mybir.SyncInfo(on_wait=[w], on_update=[]),
                        )
                        d.engine = ins.engine
                        newlist.append(d)
                    si.on_wait = [waits[-1]]
                    changed = True
                newlist.append(ins)
            if changed:
                bb.instructions = newlist


def _build(nblk):
    nc = bass.Bass()
    qT_d = nc.dram_tensor("qT", [D, LQ], BF16, kind="ExternalInput")
    kT_d = nc.dram_tensor("kT", [D, LK], BF16, kind="ExternalInput")
    v_d = nc.dram_tensor("v", [LK, D], F32, kind="ExternalInput")
    wqT_d = nc.dram_tensor("wqT", [D, H], BF16, kind="ExternalInput")
    wkT_d = nc.dram_tensor("wkT", [D, H], BF16, kind="ExternalInput")
    wv_d = nc.dram_tensor("wv", [128, 2], F32, kind="ExternalInput")
    wv8_d = nc.dram_tensor("wv8", [128, 32], FP8, kind="ExternalInput")
    vmask_d = nc.dram_tensor("vmask", [128, 2], F32, kind="ExternalInput")
    id_d = nc.dram_tensor("ident", [128, 128], F32, kind="ExternalInput")
    out_d = nc.dram_tensor("out", [LQ, D], F32, kind="ExternalOutput")

    kmaxpad = nblk * KBLK

    with TileContext(nc) as tc:
        with (
            tc.tile_pool(name="const", bufs=1) as cpool,
            tc.tile_pool(name="sums", bufs=2) as spool,
            tc.tile_pool(name="fs", bufs=3) as fpool,
            tc.tile_pool(name="ep", bufs=1) as epool,
            tc.tile_pool(name="ppj", bufs=1, space="PSUM") as ppj,
            tc.tile_pool(name="psc", bufs=1, space="PSUM") as psc,
        ):
            # ---- loads ----
            qT = [cpool.tile([128, LQ], BF16, tag=f"qT{c}", name=f"qT{c}") for c in range(2)]
            kT = [cpool.tile([128, LK], BF16, tag=f"kT{c}", name=f"kT{c}") for c in range(2)]
            wqT = [cpool.tile([128, H], BF16, tag=f"wqT{c}", name=f"wqT{c}") for c in range(2)]
            wkT = [cpool.tile([128, H], BF16, tag=f"wkT{c}", name=f"wkT{c}") for c in range(2)]
            vt = [cpool.tile([128, D], F32, tag=f"v{c}", name=f"v{c}") for c in range(2)]
            wv = cpool.tile([128, 2], F32, tag="wv", name="wv")
            wvb = cpool.tile([128, 2], BF16, tag="wvb", name="wvb")
            wv8 = cpool.tile([128, 32], FP8, tag="wv8", name="wv8")
            vmask = cpool.tile([128, 2], F32, tag="vmask", name="vmask")
            ident = cpool.tile([128, 128], F32, tag="ident", name="ident")

            for c in range(2):
                s = slice(c * 128, (c + 1) * 128)
                nc.sync.dma_start(out=qT[c][:], in_=qT_d[s, :])
                nc.sync.dma_start(out=kT[c][:], in_=kT_d[s, :])
                nc.sync.dma_start(out=wqT[c][:], in_=wqT_d[s, :])
                nc.sync.dma_start(out=wkT[c][:], in_=wkT_d[s, :])
                nc.sync.dma_start(out=vt[c][:], in_=v_d[s, :])
            nc.sync.dma_start(out=wv[:], in_=wv_d[:])
            nc.sync.dma_start(out=wv8[:], in_=wv8_d[:])
            nc.sync.dma_start(out=vmask[:], in_=vmask_d[:])
            nc.sync.dma_start(out=ident[:], in_=id_d[:])
            nc.vector.tensor_copy(out=wvb[:], in_=wv[:])

            # ---- projections: projT[h, q] with h on partitions ----
            qprojT = [cpool.tile([128, LQ], F32, tag=f"qp{c}", name=f"qp{c}") for c in range(2)]
            kprojT = [cpool.tile([128, LK], F32, tag=f"kp{c}", name=f"kp{c}") for c in range(2)]
            for hc in range(2):
                hs = slice(hc * 128, (hc + 1) * 128)
                pq = ppj.tile([128, LQ], F32, tag=f"pj{hc}", name=f"pjq{hc}")
                pk = ppj.tile([128, LK], F32, tag=f"pj{2 + hc}", name=f"pjk{hc}")
                for dc in range(2):
                    nc.tensor.matmul(
                        pq[:], lhsT=wqT[dc][:, hs], rhs=qT[dc][:],
                        start=(dc == 0), stop=(dc == 1),
                    )
                for dc in range(2):
                    nc.tensor.matmul(
                        pk[:], lhsT=wkT[dc][:, hs], rhs=kT[dc][:],
                        start=(dc == 0), stop=(dc == 1),
                    )
                nc.scalar.copy(out=qprojT[hc][:], in_=pq[:])
                nc.scalar.copy(out=kprojT[hc][:], in_=pk[:])

            # ---- main loop: scores[q, k] in PSUM (q on partitions) ----
            psqk = [psc.tile([128, LK], F32, tag=f"sqk{qc}", name=f"sqk{qc}") for qc in range(2)]
            NF = 1  # k-values per (hc, blk) fused on ACT (bias'd tanh)
            NT = KBLK - NF
            for blk in range(nblk):
                k0 = blk * KBLK
                sums = [spool.tile([128, NT * LQ], F32, tag=f"sum{c}", name=f"sum{c}") for c in range(2)]
                # F layout: column j*512 + r*256 + q  (r = h-chunk), fp8 for DoubleRow
                fts = fpool.tile([128, KBLK * 2 * LQ], BF16, tag="ft", name="ft")
                ftv = fts[:].rearrange("p (j r q) -> p j r q", r=2, q=LQ)
                for hc in range(2):
                    for j in range(NF):
                        nc.scalar.activation(
                            ftv[:, j, hc, :],
                            qprojT[hc][:],
                            AF.Tanh,
                            bias=kprojT[hc][:, k0 + j : k0 + j + 1],
                        )
                    qb = qprojT[hc][:].rearrange("p (a q) -> p a q", a=1).broadcast_to([128, NT, LQ])
                    kb = (
                        kprojT[hc][:, k0 + NF : k0 + KBLK]
                        .rearrange("p (k a) -> p k a", a=1)
                        .broadcast_to([128, NT, LQ])
                    )
                    sv = sums[hc][:].rearrange("p (k q) -> p k q", k=NT)
                    nsp = 3 if blk in (0, nblk - 1) and NT % 3 == 0 else 1
                    step = NT // nsp
                    for sp in range(nsp):
                        a, bnd = sp * step, (sp + 1) * step
                        nc.vector.tensor_add(out=sv[:, a:bnd, :], in0=qb[:, a:bnd, :], in1=kb[:, a:bnd, :])
                        nc.scalar.activation(
                            ftv[:, NF + a : NF + bnd, hc, :], sv[:, a:bnd, :], AF.Tanh
                        )
                for j in range(KBLK):
                    k = k0 + j
                    for qc in range(2):
                        for hc in range(2):
                            nc.tensor.matmul(
                                psqk[qc][:, k : k + 1],
                                lhsT=ftv[:, j, hc, qc * 128 : (qc + 1) * 128],
                                rhs=wvb[:, hc : hc + 1],
                                start=(hc == 0),
                                stop=(hc == 1),
                            )
            # zero never-written score columns (k >= kmaxpad)
            if kmaxpad < LK:
                for qc in range(2):
                    nc.vector.memset(psqk[qc][:, kmaxpad:LK], 0.0)

            # ---- transpose scores -> scoresT[k, q] ----
            sq = []
            for qc in range(2):
                t = epool.tile([128, LK], F32, tag=f"sq{qc}", name=f"sq{qc}")
                nc.scalar.copy(out=t[:, 0:128], in_=psqk[qc][:, 0:128])
                nc.scalar.copy(out=t[:, 128:LK], in_=psqk[qc][:, 128:LK])
                sq.append(t)
            pscT = [ppj.tile([128, LQ], F32, tag=f"pj{kc}", name=f"pscT{kc}") for kc in range(2)]
            for kc in range(2):
                for qc in range(2):
                    nc.tensor.transpose(
                        pscT[kc][:, qc * 128 : (qc + 1) * 128],
                        sq[qc][:, kc * 128 : (kc + 1) * 128],
                        ident[:],
                    )

            # ---- mask + softmax over q (free axis) ----
            attn = []
            for kc in range(2):
                sc = epool.tile([128, LQ], F32, tag=f"scs{kc}", name=f"scs{kc}")
                nc.vector.tensor_scalar_mul(
                    out=sc[:], in0=pscT[kc][:], scalar1=vmask[:, kc : kc + 1]
                )
                nrmax = epool.tile([128, 1], F32, tag=f"nrm{kc}", name=f"nrm{kc}")
                nc.vector.tensor_reduce(
                    out=nrmax[:], in_=sc[:], axis=AX.X, op=OP.max, negate=True
                )
                ex = epool.tile([128, LQ], F32, tag=f"ex{kc}", name=f"ex{kc}")
                rsum = epool.tile([128, 1], F32, tag=f"rs{kc}", name=f"rs{kc}")
                nc.scalar.activation(
                    ex[:], sc[:], AF.Exp, bias=nrmax[:, 0:1], scale=1.0,
                    accum_out=rsum[:, 0:1],
                )
                rinv = epool.tile([128, 1], F32, tag=f"ri{kc}", name=f"ri{kc}")
                nc.vector.reciprocal(out=rinv[:], in_=rsum[:])
                at = epool.tile([128, LQ], F32, tag=f"at{kc}", name=f"at{kc}")
                nc.vector.tensor_scalar_mul(
                    out=at[:], in0=ex[:], scalar1=rinv[:, 0:1]
                )
                attn.append(at)

            # ---- out[q, d] = sum_k attn[k, q] * v[k, d] ----
            for qc in range(2):
                po = ppj.tile([128, D], F32, tag=f"pj{2 + qc}", name=f"po{qc}")
                for kc in range(2):
                    nc.tensor.matmul(
                        po[:],
                        lhsT=attn[kc][:, qc * 128 : (qc + 1) * 128],
                        rhs=vt[kc][:],
                        start=(kc == 0),
                        stop=(kc == 1),
                    )
                ot = epool.tile([128, D], F32, tag=f"ot{qc}", name=f"ot{qc}")
                nc.scalar.copy(out=ot[:], in_=po[:])
                nc.sync.dma_start(out=out_d[qc * 128 : (qc + 1) * 128, :], in_=ot[:])

    _split_multiwait(nc)
    return nc


def kernel(queries, keyes, values, valid_lens, W_q, W_k, W_v):
    queries = np.asarray(queries, dtype=np.float32)
    keyes = np.asarray(keyes, dtype=np.float32)
    values = np.asarray(values, dtype=np.float32)
    valid = np.asarray(valid_lens).astype(np.int64)
    W_q = np.asarray(W_q, dtype=np.float32)
    W_k = np.asarray(W_k, dtype=np.float32)
    W_v = np.asarray(W_v, dtype=np.float32)

    kmax = int(valid.max())
    nblk = max(1, -(-kmax // KBLK))
    nblk = min(nblk, LK // KBLK)
    nc = _build(nblk)

    import ml_dtypes

    bf16 = ml_dtypes.bfloat16
    wqT = np.ascontiguousarray(W_q.T).astype(bf16)  # [D, H]
    wkT = np.ascontiguousarray(W_k.T).astype(bf16)
    wv2 = np.ascontiguousarray(W_v[0].reshape(2, 128).T)  # [128, 2]
    wv8 = np.zeros((128, 32), dtype=ml_dtypes.float8_e4m3)
    wv8[:, 0] = W_v[0][0:128].astype(ml_dtypes.float8_e4m3)
    wv8[:, 16] = W_v[0][128:256].astype(ml_dtypes.float8_e4m3)
    ident = np.eye(128, dtype=np.float32)

    in_maps = []
    for b in range(B):
        mask = (np.arange(LK) < valid[b]).astype(np.float32)
        in_maps.append(
            {
                "qT": np.ascontiguousarray(queries[b].T).astype(bf16),
                "kT": np.ascontiguousarray(keyes[b].T).astype(bf16),
                "v": np.ascontiguousarray(values[b]),
                "wqT": wqT,
                "wkT": wkT,
                "wv": wv2,
                "wv8": wv8,
                "vmask": np.ascontiguousarray(mask.reshape(2, 128).T),
                "ident": ident,
            }
        )

    res = run_bass_kernel_spmd(nc, in_maps, core_ids=list(range(B)))
    return np.stack([res.results[b]["out"] for b in range(B)], axis=0)
